# revision 9
# baseline (speedup 1.0000x reference)
"""Trainium2 Bass kernel for nn_AttributeEmbeddingLayer (gnn_message_passing).

Two-phase heterogeneous GNN attention layer on 8 NeuronCores:
  phase 1: user rows attend over product embeddings (user_nbrs)
  phase 2: product rows attend over the UPDATED user embeddings (product_nbrs)

Distribution: data-parallel over the node dimension (1024 rows/core), small
parameter tensors replicated, the other-type embedding table replicated
(phase 2's table is produced on-device via AllGather); the Beta reduction is
a cross-device AllReduce of 4 partial sums.

Fast path exploits the neighbor-list structure (the K=32 neighbor indices of
every node share one residue r mod (N/K), i.e. they are exactly the rows
{r + 256*u}): tables are re-laid out on device so each node's 32 neighbor
rows form ONE contiguous block, gathered with a single-index-per-partition
indirect DMA (the only gather shape TRN2 hardware supports efficiently).
The structure is verified on the host; inputs without it fall back to a
numpy implementation of the same math.
"""

import numpy as np

# ---------------------------------------------------------------- constants
N_NODES = 8192      # nodes per type (users == products == 8192)
E = 128             # embedding dim
D = 64              # attention dim
K = 32              # neighbors per (metapath, node)
M = 4               # metapaths
CORES = 8
NLOC = N_NODES // CORES          # 1024 rows per core
NB = NLOC // 128                 # 8 n-blocks of 128 rows per core
STRIDE = N_NODES // K            # 256; neighbor sets are {r + STRIDE*u}
NRES = STRIDE

FP = None  # mybir.dt.float32, set lazily


# ---------------------------------------------------------------- host math
def _phase_np(src, other, nbrs, v, x, w, b, wq, bq, q):
    """Numpy port of the reference _phase (used as fallback / verification)."""
    m, n, k = nbrs.shape
    n_other = other.shape[0]
    out = src.copy()
    beta_raw = np.zeros(m, np.float32)
    H_all = np.empty((m, n, src.shape[1]), np.float32)
    baseline = np.where(np.arange(m) == 0, np.float32(-1e-9),
                        np.float32(1.0) / n_other).astype(np.float32)
    for mi in range(m):
        agg = np.empty((n, src.shape[1]), np.float32)
        CH = 1024
        for s in range(0, n, CH):
            sl = slice(s, s + CH)
            nbr = other[nbrs[mi, sl]]                      # [CH,K,E]
            ps = src[sl] @ v[mi]                          # [CH,D]
            pn = nbr @ w[mi]                              # [CH,K,D]
            h = np.tanh(ps[:, None, :] + pn + b[mi][None, None, :])
            sc = h @ x[mi, 0]                             # [CH,K]
            mx = np.maximum(sc.max(-1), baseline[mi])
            e = np.exp(sc - mx[:, None])
            den = e.sum(-1) + (n_other - k) * np.exp(baseline[mi] - mx)
            A = e / den[:, None]
            agg[sl] = np.einsum('nk,nke->ne', A, nbr)
        H = src + agg
        H_all[mi] = H
        sem = np.tanh(H @ wq[mi] + bq[mi][None, :])
        beta_raw[mi] = (sem @ q[mi, 0]).mean()
    eb = np.exp(beta_raw - beta_raw.max())
    beta = eb / eb.sum()
    return np.einsum('m,mne->ne', beta, H_all).astype(np.float32)


def _reference_np(user, product, V, X, W_p, B_p, W_q, B_q, Q,
                  user_nbrs, product_nbrs):
    user_out = _phase_np(user, product, user_nbrs,
                         V[0], X[0], W_p[0], B_p[0], W_q[0], B_q[0], Q[0])
    product_out = _phase_np(product, user_out, product_nbrs,
                            V[1], X[1], W_p[1], B_p[1], W_q[1], B_q[1], Q[1])
    return (user_out, product_out)


def _check_structured(nbrs):
    """True iff every (m, n) neighbor set is exactly {r + STRIDE*u, u=0..K-1}."""
    if nbrs.shape != (M, N_NODES, K):
        return False
    r = nbrs[:, :, 0] % STRIDE
    want = r[:, :, None] + STRIDE * np.arange(K, dtype=nbrs.dtype)[None, None, :]
    return bool(np.array_equal(np.sort(nbrs, axis=-1), np.sort(want, axis=-1)))


# ---------------------------------------------------------------- device IR
_CACHE = {}


def _build_graph():
    import sys
    if "/opt/trn_rl_repo" not in sys.path:
        sys.path.insert(0, "/opt/trn_rl_repo")
    import concourse.bass as bass
    import concourse.bacc as bacc
    import concourse.mybir as mybir
    import concourse.tile as tile

    fp = mybir.dt.float32
    i32 = mybir.dt.int32
    AF = mybir.ActivationFunctionType
    ALU = mybir.AluOpType
    AX = mybir.AxisListType

    nc = bacc.Bacc("TRN2", target_bir_lowering=False, num_devices=CORES)

    # ---------------- I/O -------------------------------------------------
    t_user = nc.dram_tensor("user_shard", [NLOC, E], fp, kind="ExternalInput")
    t_prod_shard = nc.dram_tensor("product_shard", [NLOC, E], fp, kind="ExternalInput")
    t_prod_full = nc.dram_tensor("product_full", [N_NODES, E], fp, kind="ExternalInput")
    t_V = nc.dram_tensor("V_w", [2, M, E, D], fp, kind="ExternalInput")
    t_Wp = nc.dram_tensor("Wp_w", [2, M, E, D], fp, kind="ExternalInput")
    t_Wq = nc.dram_tensor("Wq_w", [2, M, E, D], fp, kind="ExternalInput")
    # host-replicated across 128 partitions, m-concat along free dim:
    t_Xrep = nc.dram_tensor("Xrep", [2, 128, M * D], fp, kind="ExternalInput")
    t_Brep = nc.dram_tensor("Brep", [2, 128, M * D], fp, kind="ExternalInput")
    t_Bq = nc.dram_tensor("Bq_w", [2, M, D], fp, kind="ExternalInput")
    t_Q = nc.dram_tensor("Q_w", [2, M, 1, D], fp, kind="ExternalInput")
    t_r1 = nc.dram_tensor("r_user", [M, NLOC], i32, kind="ExternalInput")
    t_r2 = nc.dram_tensor("r_prod", [M, NLOC], i32, kind="ExternalInput")
    t_eye = nc.dram_tensor("eye128", [128, 128], fp, kind="ExternalInput")

    t_uout = nc.dram_tensor("user_out_shard", [NLOC, E], fp, kind="ExternalOutput")
    t_pout = nc.dram_tensor("product_out_shard", [NLOC, E], fp, kind="ExternalOutput")

    # softmax baseline constants (match reference semantics without max-sub)
    CB = [float((N_NODES - K) * np.exp(np.float32(-1e-9)))] + \
         [float((N_NODES - K) * np.exp(np.float32(1.0) / N_NODES))] * (M - 1)

    with tile.TileContext(nc) as tc:
        with (
            tc.tile_pool(name="wpool", bufs=1) as wp,
            tc.tile_pool(name="spool", bufs=1) as sp,
            tc.tile_pool(name="mpool", bufs=2) as mp,
            tc.tile_pool(name="aggpool", bufs=1) as agp,
            tc.tile_pool(name="psum", bufs=3, space="PSUM") as pp,
            tc.tile_pool(name="pbeta", bufs=1, space="PSUM") as pb,
            tc.tile_pool(name="dram", bufs=1, space="DRAM") as dp,
        ):
            # ---------------- persistent weights -------------------------
            eye = wp.tile([128, 128], fp, name="eye")
            nc.sync.dma_start(eye[:], t_eye[:])
            ones_r = wp.tile([1, 128], fp, name="ones_r")
            nc.vector.memset(ones_r[:], 1.0)

            Vw, Wpw, Wqw, BqT, qT = {}, {}, {}, {}, {}
            for ph in range(2):
                for m in range(M):
                    Vw[ph, m] = wp.tile([E, D], fp, name=f"V_{ph}_{m}")
                    nc.sync.dma_start(Vw[ph, m][:], t_V[ph, m])
                    Wpw[ph, m] = wp.tile([E, D], fp, name=f"Wp_{ph}_{m}")
                    nc.sync.dma_start(Wpw[ph, m][:], t_Wp[ph, m])
                    Wqw[ph, m] = wp.tile([E, D], fp, name=f"Wq_{ph}_{m}")
                    nc.sync.dma_start(Wqw[ph, m][:], t_Wq[ph, m])
                    BqT[ph, m] = wp.tile([D, 1], fp, name=f"BqT_{ph}_{m}")
                    nc.sync.dma_start(BqT[ph, m][:], t_Bq[ph, m, :, None])
                    qT[ph, m] = wp.tile([D, 1], fp, name=f"qT_{ph}_{m}")
                    nc.sync.dma_start(qT[ph, m][:], t_Q[ph, m, 0, :, None])
            x_all, b_all = {}, {}
            for ph in range(2):
                x_all[ph] = wp.tile([128, M * D], fp, name=f"xall_{ph}")
                nc.sync.dma_start(x_all[ph][:], t_Xrep[ph])
                b_all[ph] = wp.tile([128, M * D], fp, name=f"ball_{ph}")
                nc.sync.dma_start(b_all[ph][:], t_Brep[ph])

            # ---------------- internal DRAM ------------------------------
            t5e = dp.tile([N_NODES, E], fp, name="t5e")
            t5pw = [dp.tile([N_NODES, D], fp, name=f"t5pw_{m}") for m in range(M)]
            ag_in = dp.tile([NLOC, E], fp, name="ag_in")
            shared = "Shared" if CORES > 4 else "Local"
            ag_out = dp.tile([N_NODES, E], fp, name="ag_out", addr_space=shared)


            def t5_dst(t5ap, a, width):
                """AP for writing source rows [128a, 128a+128) of a table into
                its T5 layout ([r, u] -> flat row r*K + u)."""
                v = t5ap[:].rearrange("(r u) e -> r u e", u=K)
                if STRIDE >= 128:
                    r0 = (128 * a) % STRIDE
                    u0 = (128 * a) // STRIDE
                    return v[r0:r0 + 128, u0, :]
                # shrunk configs: 128 rows span several u slots
                g = 128 // STRIDE
                u0 = (128 * a) // STRIDE
                return v[:, u0:u0 + g, :].rearrange("r g e -> g r e")

            def emit_phase(ph, src_dram, other_dram, r_dram, out_drams):
                # ---- src tiles + srcT --------------------------------------
                src_sb = []
                srcT = sp.tile([128, NLOC], fp, name=f"srcT_{ph}", tag="srcT")
                for nb in range(NB):
                    st = sp.tile([128, E], fp, name=f"src_{ph}_{nb}", tag=f"src{nb}")
                    nc.sync.dma_start(st[:], src_dram[nb * 128:(nb + 1) * 128, :])
                    src_sb.append(st)
                    pt = pp.tile([128, 128], fp, name=f"pt_{ph}_{nb}", tag="pmain", space="PSUM")
                    nc.tensor.transpose(pt[:], st[:], eye[:])
                    nc.scalar.copy(srcT[:, nb * 128:(nb + 1) * 128], pt[:])

                # ---- T5 tables + PW ---------------------------------------
                for a in range(N_NODES // 128):
                    ot = mp.tile([128, E], fp, name=f"ot_{ph}_{a}", tag="ot")
                    nc.sync.dma_start(ot[:], other_dram[128 * a:128 * (a + 1), :])
                    nc.sync.dma_start(t5_dst(t5e, a, E), ot[:])
                    ptr = pp.tile([128, 128], fp, name=f"potT_{ph}_{a}", tag="pmain", space="PSUM")
                    nc.tensor.transpose(ptr[:], ot[:], eye[:])
                    otT = mp.tile([128, 128], fp, name=f"otT_{ph}_{a}", tag="otT")
                    nc.scalar.copy(otT[:], ptr[:])
                    ppw = pp.tile([128, M * D], fp, name=f"ppw_{ph}_{a}", tag="pmain", space="PSUM")
                    for m in range(M):
                        nc.tensor.matmul(ppw[:, m * D:(m + 1) * D], lhsT=otT[:],
                                         rhs=Wpw[ph, m][:], start=True, stop=True)
                    pwt = mp.tile([128, M * D], fp, name=f"pw_{ph}_{a}", tag="pwt")
                    nc.vector.tensor_copy(pwt[:], ppw[:])
                    for m in range(M):
                        nc.sync.dma_start(t5_dst(t5pw[m], a, D),
                                          pwt[:, m * D:(m + 1) * D])

                # ---- S' = src @ V + b  ([128, nb, m, D] in SBUF) -----------
                spr = sp.tile([128, NB * M * D], fp, name=f"spr_{ph}", tag="spr")
                for nb in range(NB):
                    psp = pp.tile([128, M * D], fp, name=f"psp_{ph}_{nb}", tag="pmain", space="PSUM")
                    for m in range(M):
                        nc.tensor.matmul(psp[:, m * D:(m + 1) * D],
                                         lhsT=srcT[:, nb * 128:(nb + 1) * 128],
                                         rhs=Vw[ph, m][:], start=True, stop=True)
                    nc.vector.tensor_tensor(
                        out=spr[:, nb * M * D:(nb + 1) * M * D],
                        in0=psp[:], in1=b_all[ph][:], op=ALU.add)

                # ---- main loop --------------------------------------------
                pbeta = [pb.tile([1, 128], fp, name=f"pbeta_{ph}_{m}",
                                 tag=f"pbeta{m}", space="PSUM") for m in range(M)]
                aggs = {}
                t5e_v = t5e[:].rearrange("(r u) e -> r (u e)", u=K)
                for nb in range(NB):
                    for m in range(M):
                        t5pw_v = t5pw[m][:].rearrange("(r u) d -> r (u d)", u=K)
                        rt = mp.tile([128, 1], i32, name=f"r_{ph}_{nb}_{m}", tag="rt")
                        nc.sync.dma_start(rt[:], r_dram[m, nb * 128:(nb + 1) * 128, None])
                        gpw = mp.tile([128, K * D], fp, name=f"gpw_{ph}_{nb}_{m}", tag="gpw")
                        nc.gpsimd.indirect_dma_start(
                            out=gpw[:], out_offset=None, in_=t5pw_v,
                            in_offset=bass.IndirectOffsetOnAxis(ap=rt[:, :1], axis=0))
                        gemb = mp.tile([128, K * E], fp, name=f"ge_{ph}_{nb}_{m}", tag="gemb")
                        nc.gpsimd.indirect_dma_start(
                            out=gemb[:], out_offset=None, in_=t5e_v,
                            in_offset=bass.IndirectOffsetOnAxis(ap=rt[:, :1], axis=0))

                        spm = spr[:, (nb * M + m) * D:(nb * M + m + 1) * D]
                        h = mp.tile([128, K * D], fp, name=f"h_{ph}_{nb}_{m}", tag="h")
                        h3 = h[:].rearrange("p (k d) -> p k d", d=D)
                        nc.vector.tensor_tensor(
                            out=h3, in0=gpw[:].rearrange("p (k d) -> p k d", d=D),
                            in1=spm[:, None, :].to_broadcast([128, K, D]), op=ALU.add)
                        nc.scalar.activation(h3, h3, AF.Tanh)
                        xm = x_all[ph][:, m * D:(m + 1) * D]
                        nc.vector.tensor_tensor(
                            out=h3, in0=h3,
                            in1=xm[:, None, :].to_broadcast([128, K, D]), op=ALU.mult)
                        sc = mp.tile([128, K], fp, name=f"sc_{ph}_{nb}_{m}", tag="sc")
                        nc.vector.tensor_reduce(sc[:], h3, axis=AX.X, op=ALU.add)
                        esc = mp.tile([128, K], fp, name=f"esc_{ph}_{nb}_{m}", tag="esc")
                        den = mp.tile([128, 1], fp, name=f"den_{ph}_{nb}_{m}", tag="den")
                        nc.scalar.activation(esc[:], sc[:], AF.Exp, accum_out=den[:])
                        nc.vector.tensor_scalar_add(den[:], den[:], CB[m])
                        rin = mp.tile([128, 1], fp, name=f"rin_{ph}_{nb}_{m}", tag="rin")
                        nc.vector.reciprocal(rin[:], den[:])
                        att = mp.tile([128, K], fp, name=f"att_{ph}_{nb}_{m}", tag="att")
                        nc.vector.tensor_scalar_mul(att[:], esc[:], rin[:, :1])

                        # agg = sum_u A[:,u] * emb[:,u,:]  -> [128, E]
                        wemb = mp.tile([128, E * K], fp, name=f"we_{ph}_{nb}_{m}", tag="wemb")
                        we3 = wemb[:].rearrange("p (e k) -> p k e", k=K)
                        nc.vector.tensor_tensor(
                            out=we3, in0=gemb[:].rearrange("p (k e) -> p k e", e=E),
                            in1=att[:, :, None].to_broadcast([128, K, E]), op=ALU.mult)
                        agg = agp.tile([128, E], fp, name=f"agg_{ph}_{nb}_{m}",
                                       tag=f"agg{nb}_{m}")
                        nc.vector.tensor_reduce(
                            agg[:], wemb[:].rearrange("p (e k) -> p e k", k=K),
                            axis=AX.X, op=ALU.add)
                        aggs[nb, m] = agg

                        # sem path: semT = tanh(Wq^T @ (srcT + aggT) + BqT)
                        pat = pp.tile([128, 128], fp, name=f"pat_{ph}_{nb}_{m}", tag="pmain", space="PSUM")
                        nc.tensor.transpose(pat[:], agg[:], eye[:])
                        aggT = mp.tile([128, 128], fp, name=f"at_{ph}_{nb}_{m}", tag="aggT")
                        nc.scalar.copy(aggT[:], pat[:])
                        psem = pp.tile([D, 128], fp, name=f"ps_{ph}_{nb}_{m}", tag="pmain", space="PSUM")
                        nc.tensor.matmul(psem[:], lhsT=Wqw[ph, m][:],
                                         rhs=srcT[:, nb * 128:(nb + 1) * 128],
                                         start=True, stop=False)
                        nc.tensor.matmul(psem[:], lhsT=Wqw[ph, m][:], rhs=aggT[:],
                                         start=False, stop=True)
                        semT = mp.tile([D, 128], fp, name=f"st_{ph}_{nb}_{m}", tag="semT")
                        nc.scalar.activation(semT[:], psem[:], AF.Tanh, bias=BqT[ph, m][:, :1])
                        nc.tensor.matmul(pbeta[m][:], lhsT=qT[ph, m][:], rhs=semT[:],
                                         start=(nb == 0), stop=(nb == NB - 1))

                # ---- beta (AllReduce of partial means, then softmax) -------
                ar_in = dp.tile([1, 8], fp, name=f"ar_in_{ph}")
                ar_out = dp.tile([1, 8], fp, name=f"ar_out_{ph}", addr_space=shared)
                braw = mp.tile([1, 8], fp, name=f"braw_{ph}", tag="braw")
                nc.vector.memset(braw[:], 0.0)
                for m in range(M):
                    nc.vector.tensor_reduce(braw[:, m:m + 1], pbeta[m][:],
                                            axis=AX.X, op=ALU.add)
                nc.vector.tensor_scalar_mul(braw[:], braw[:], 1.0 / N_NODES)
                nc.gpsimd.dma_start(ar_in[:], braw[:])
                nc.gpsimd.collective_compute(
                    "AllReduce", ALU.add,
                    replica_groups=[list(range(CORES))],
                    ins=[ar_in.opt()], outs=[ar_out.opt()])
                brg = mp.tile([1, 8], fp, name=f"brg_{ph}", tag="brg")
                nc.sync.dma_start(brg[:], ar_out[:])
                eb = mp.tile([1, M], fp, name=f"eb_{ph}", tag="eb")
                ebs = mp.tile([1, 1], fp, name=f"ebs_{ph}", tag="ebs")
                nc.scalar.activation(eb[:], brg[:, :M], AF.Exp, accum_out=ebs[:])
                ebr = mp.tile([1, 1], fp, name=f"ebr_{ph}", tag="ebr")
                nc.vector.reciprocal(ebr[:], ebs[:])
                beta = mp.tile([1, M], fp, name=f"beta_{ph}", tag="beta")
                nc.vector.tensor_scalar_mul(beta[:], eb[:], ebr[:, :1])
                pbb = pp.tile([128, M], fp, name=f"pbb_{ph}", tag="pmain", space="PSUM")
                nc.tensor.matmul(pbb[:], lhsT=ones_r[:], rhs=beta[:], start=True, stop=True)
                beta_bc = mp.tile([128, M], fp, name=f"bbc_{ph}", tag="bbc")
                nc.vector.tensor_copy(beta_bc[:], pbb[:])

                # ---- out = src + sum_m beta_m * agg_m ----------------------
                for nb in range(NB):
                    out_t = mp.tile([128, E], fp, name=f"out_{ph}_{nb}", tag="outt")
                    tmp_t = mp.tile([128, E], fp, name=f"tmp_{ph}_{nb}", tag="tmpt")
                    nc.vector.tensor_scalar_mul(out_t[:], aggs[nb, 0][:], beta_bc[:, 0:1])
                    nc.vector.tensor_tensor(out=out_t[:], in0=out_t[:],
                                            in1=src_sb[nb][:], op=ALU.add)
                    for m in range(1, M):
                        nc.vector.tensor_scalar_mul(tmp_t[:], aggs[nb, m][:],
                                                    beta_bc[:, m:m + 1])
                        nc.vector.tensor_tensor(out=out_t[:], in0=out_t[:],
                                                in1=tmp_t[:], op=ALU.add)
                    for od in out_drams:
                        nc.sync.dma_start(od[nb * 128:(nb + 1) * 128, :], out_t[:])

            # ================= phase 1: users ============================
            emit_phase(0, t_user, t_prod_full, t_r1, [t_uout, ag_in])
            nc.gpsimd.collective_compute(
                "AllGather", mybir.AluOpType.bypass,
                replica_groups=[list(range(CORES))],
                ins=[ag_in.opt()], outs=[ag_out.opt()])
            # ================= phase 2: products =========================
            emit_phase(1, t_prod_shard, ag_out, t_r2, [t_pout])

    nc.compile()
    return nc


def _get_graph():
    if "nc" not in _CACHE:
        _CACHE["nc"] = _build_graph()
    return _CACHE["nc"]


# ---------------------------------------------------------------- runner
def _get_runner():
    """Build (once) a cached jitted SPMD executable for the graph.

    Mirrors concourse.bass2jax.run_bass_via_pjrt's multi-core path but keeps
    the jitted function so repeated kernel() calls don't retrace/recompile,
    and exposes device-resident timing.
    """
    if "runner" in _CACHE:
        return _CACHE["runner"]
    import sys
    if "/opt/trn_rl_repo" not in sys.path:
        sys.path.insert(0, "/opt/trn_rl_repo")
    import jax
    import numpy as _np
    from jax.experimental.shard_map import shard_map
    from jax.sharding import Mesh, PartitionSpec
    from concourse import bass2jax, mybir

    nc = _get_graph()
    bass2jax.install_neuronx_cc_hook()
    assert nc.dbg_addr is None
    pid_name = nc.partition_id_tensor.name if nc.partition_id_tensor else None

    in_names, out_names, out_avals = [], [], []
    for alloc in nc.m.functions[0].allocations:
        if not isinstance(alloc, mybir.MemoryLocationSet):
            continue
        name = alloc.memorylocations[0].name
        if alloc.kind == "ExternalInput":
            if name != pid_name:
                in_names.append(name)
        elif alloc.kind == "ExternalOutput":
            out_names.append(name)
            out_avals.append(jax.core.ShapedArray(
                tuple(alloc.tensor_shape), mybir.dt.np(alloc.dtype)))
    n_params = len(in_names)
    all_names = in_names + out_names
    if pid_name is not None:
        all_names = all_names + [pid_name]

    def _body(*args):
        operands = list(args)
        if pid_name is not None:
            operands.append(bass2jax.partition_id_tensor())
        outs = bass2jax._bass_exec_p.bind(
            *operands, out_avals=tuple(out_avals), in_names=tuple(all_names),
            out_names=tuple(out_names), lowering_input_output_aliases=(),
            sim_require_finite=True, sim_require_nnan=True, nc=nc)
        return tuple(outs)

    devices = jax.devices()[:CORES]
    mesh = Mesh(_np.asarray(devices), ("core",))
    n_outs = len(out_names)
    in_specs = (PartitionSpec("core"),) * (n_params + n_outs)
    out_specs = (PartitionSpec("core"),) * n_outs
    donate = tuple(range(n_params, n_params + n_outs))
    sharded = jax.jit(
        shard_map(_body, mesh=mesh, in_specs=in_specs, out_specs=out_specs,
                  check_rep=False),
        donate_argnums=donate, keep_unused=True)

    runner = dict(fn=sharded, in_names=in_names, out_names=out_names,
                  out_avals=out_avals, mesh=mesh)
    _CACHE["runner"] = runner
    return runner


def _run_spmd(in_maps, timeit=0):
    """Run the SPMD graph; returns (per-core results list, best_step_ns|None)."""
    import jax
    import jax.numpy as jnp
    import numpy as _np
    import time as _time
    from jax.sharding import NamedSharding, PartitionSpec

    r = _get_runner()
    fn, in_names, out_names, out_avals = \
        r["fn"], r["in_names"], r["out_names"], r["out_avals"]
    mesh = r["mesh"]

    concat_in = [_np.concatenate([_np.asarray(in_maps[c][k]) for c in range(CORES)],
                                 axis=0) for k in in_names]
    sharding = NamedSharding(mesh, PartitionSpec("core"))
    dev_in = [jax.device_put(a, sharding) for a in concat_in]

    def zeros():
        return [jax.device_put(
            _np.zeros((CORES * av.shape[0], *av.shape[1:]), av.dtype), sharding)
            for av in out_avals]

    outs = fn(*dev_in, *zeros())
    jax.block_until_ready(outs)
    best_ns = None
    if timeit:
        times = []
        for _ in range(timeit):
            z = zeros()
            jax.block_until_ready(z)
            t0 = _time.perf_counter()
            outs2 = fn(*dev_in, *z)
            jax.block_until_ready(outs2)
            times.append(_time.perf_counter() - t0)
        best_ns = int(min(times) * 1e9)
        outs = outs2
    np_outs = [_np.asarray(o) for o in outs]
    results = [{name: np_outs[i].reshape(CORES, *out_avals[i].shape)[c]
                for i, name in enumerate(out_names)} for c in range(CORES)]
    return results, best_ns


def _make_in_maps(user, product, V, X, W_p, B_p, W_q, B_q, Q,
                  user_nbrs, product_nbrs):
    Xrep = np.ascontiguousarray(
        np.broadcast_to(X[:, :, 0, :][:, None, :, :], (2, 128, M, D))
        .reshape(2, 128, M * D)).astype(np.float32)
    Brep = np.ascontiguousarray(
        np.broadcast_to(B_p[:, None, :, :], (2, 128, M, D))
        .reshape(2, 128, M * D)).astype(np.float32)
    r_user = (user_nbrs[:, :, 0] % STRIDE).astype(np.int32)
    r_prod = (product_nbrs[:, :, 0] % STRIDE).astype(np.int32)
    eye = np.eye(128, dtype=np.float32)
    in_maps = []
    for c in range(CORES):
        rows = slice(c * NLOC, (c + 1) * NLOC)
        in_maps.append({
            "user_shard": user[rows],
            "product_shard": product[rows],
            "product_full": product,
            "V_w": V, "Wp_w": W_p, "Wq_w": W_q,
            "Xrep": Xrep, "Brep": Brep,
            "Bq_w": B_q, "Q_w": Q,
            "r_user": np.ascontiguousarray(r_user[:, rows]),
            "r_prod": np.ascontiguousarray(r_prod[:, rows]),
            "eye128": eye,
        })
    return in_maps


# ---------------------------------------------------------------- entry
def kernel(user, product, V, X, W_p, B_p, W_q, B_q, Q, user_nbrs, product_nbrs):
    user = np.asarray(user, np.float32)
    product = np.asarray(product, np.float32)
    V = np.asarray(V, np.float32)
    X = np.asarray(X, np.float32)
    W_p = np.asarray(W_p, np.float32)
    B_p = np.asarray(B_p, np.float32)
    W_q = np.asarray(W_q, np.float32)
    B_q = np.asarray(B_q, np.float32)
    Q = np.asarray(Q, np.float32)
    user_nbrs = np.asarray(user_nbrs)
    product_nbrs = np.asarray(product_nbrs)

    if not (_check_structured(user_nbrs) and _check_structured(product_nbrs)):
        # General-index fallback: same math on the host.
        return _reference_np(user, product, V, X, W_p, B_p, W_q, B_q, Q,
                             user_nbrs, product_nbrs)

    in_maps = _make_in_maps(user, product, V, X, W_p, B_p, W_q, B_q, Q,
                            user_nbrs, product_nbrs)
    results, _ = _run_spmd(in_maps)
    user_out = np.concatenate([results[c]["user_out_shard"]
                               for c in range(CORES)], axis=0)
    product_out = np.concatenate([results[c]["product_out_shard"]
                                  for c in range(CORES)], axis=0)
    return (user_out, product_out)


# revision 12
# speedup vs baseline: 7.1042x; 7.1042x over previous
"""Trainium2 Bass kernel for nn_AttributeEmbeddingLayer (gnn_message_passing).

Two-phase heterogeneous GNN attention layer on 8 NeuronCores:
  phase 1: user rows attend over product embeddings (user_nbrs)
  phase 2: product rows attend over the UPDATED user embeddings (product_nbrs)

Distribution: data-parallel over the node dimension (1024 rows/core), small
parameter tensors replicated, the other-type embedding table replicated
(phase 2's table is produced on-device via AllGather); the Beta reduction is
a cross-device AllReduce of 4 partial sums.

Fast path exploits the neighbor-list structure (the K=32 neighbor indices of
every node share one residue r mod (N/K), i.e. they are exactly the rows
{r + 256*u}): tables are re-laid out on device so each node's 32 neighbor
rows form ONE contiguous block, gathered with a single-index-per-partition
indirect DMA (the only gather shape TRN2 hardware supports efficiently).
The structure is verified on the host; inputs without it fall back to a
numpy implementation of the same math.
"""

import numpy as np

# ---------------------------------------------------------------- constants
N_NODES = 8192      # nodes per type (users == products == 8192)
E = 128             # embedding dim
D = 64              # attention dim
K = 32              # neighbors per (metapath, node)
M = 4               # metapaths
CORES = 8
NLOC = N_NODES // CORES          # 1024 rows per core
NB = NLOC // 128                 # 8 n-blocks of 128 rows per core
STRIDE = N_NODES // K            # 256; neighbor sets are {r + STRIDE*u}
NRES = STRIDE

FP = None  # mybir.dt.float32, set lazily
VARIANT = "full"  # "full" | "gather_only" | "no_gather"  (perf bisection)


# ---------------------------------------------------------------- host math
def _phase_np(src, other, nbrs, v, x, w, b, wq, bq, q):
    """Numpy port of the reference _phase (used as fallback / verification)."""
    m, n, k = nbrs.shape
    n_other = other.shape[0]
    out = src.copy()
    beta_raw = np.zeros(m, np.float32)
    H_all = np.empty((m, n, src.shape[1]), np.float32)
    baseline = np.where(np.arange(m) == 0, np.float32(-1e-9),
                        np.float32(1.0) / n_other).astype(np.float32)
    for mi in range(m):
        agg = np.empty((n, src.shape[1]), np.float32)
        CH = 1024
        for s in range(0, n, CH):
            sl = slice(s, s + CH)
            nbr = other[nbrs[mi, sl]]                      # [CH,K,E]
            ps = src[sl] @ v[mi]                          # [CH,D]
            pn = nbr @ w[mi]                              # [CH,K,D]
            h = np.tanh(ps[:, None, :] + pn + b[mi][None, None, :])
            sc = h @ x[mi, 0]                             # [CH,K]
            mx = np.maximum(sc.max(-1), baseline[mi])
            e = np.exp(sc - mx[:, None])
            den = e.sum(-1) + (n_other - k) * np.exp(baseline[mi] - mx)
            A = e / den[:, None]
            agg[sl] = np.einsum('nk,nke->ne', A, nbr)
        H = src + agg
        H_all[mi] = H
        sem = np.tanh(H @ wq[mi] + bq[mi][None, :])
        beta_raw[mi] = (sem @ q[mi, 0]).mean()
    eb = np.exp(beta_raw - beta_raw.max())
    beta = eb / eb.sum()
    return np.einsum('m,mne->ne', beta, H_all).astype(np.float32)


def _reference_np(user, product, V, X, W_p, B_p, W_q, B_q, Q,
                  user_nbrs, product_nbrs):
    user_out = _phase_np(user, product, user_nbrs,
                         V[0], X[0], W_p[0], B_p[0], W_q[0], B_q[0], Q[0])
    product_out = _phase_np(product, user_out, product_nbrs,
                            V[1], X[1], W_p[1], B_p[1], W_q[1], B_q[1], Q[1])
    return (user_out, product_out)


def _check_structured(nbrs):
    """True iff every (m, n) neighbor set is exactly {r + STRIDE*u, u=0..K-1}."""
    if nbrs.shape != (M, N_NODES, K):
        return False
    r = nbrs[:, :, 0] % STRIDE
    want = r[:, :, None] + STRIDE * np.arange(K, dtype=nbrs.dtype)[None, None, :]
    return bool(np.array_equal(np.sort(nbrs, axis=-1), np.sort(want, axis=-1)))


# ---------------------------------------------------------------- device IR
_CACHE = {}


def _build_graph():
    import sys
    if "/opt/trn_rl_repo" not in sys.path:
        sys.path.insert(0, "/opt/trn_rl_repo")
    import concourse.bass as bass
    import concourse.bacc as bacc
    import concourse.mybir as mybir
    import concourse.tile as tile

    fp = mybir.dt.float32
    i32 = mybir.dt.int32
    AF = mybir.ActivationFunctionType
    ALU = mybir.AluOpType
    AX = mybir.AxisListType

    nc = bacc.Bacc("TRN2", target_bir_lowering=False, num_devices=CORES)

    # ---------------- I/O -------------------------------------------------
    t_user = nc.dram_tensor("user_shard", [NLOC, E], fp, kind="ExternalInput")
    t_prod_shard = nc.dram_tensor("product_shard", [NLOC, E], fp, kind="ExternalInput")
    t_prod_full = nc.dram_tensor("product_full", [N_NODES, E], fp, kind="ExternalInput")
    t_V = nc.dram_tensor("V_w", [2, M, E, D], fp, kind="ExternalInput")
    t_Wp = nc.dram_tensor("Wp_w", [2, M, E, D], fp, kind="ExternalInput")
    t_Wq = nc.dram_tensor("Wq_w", [2, M, E, D], fp, kind="ExternalInput")
    # host-replicated across 128 partitions, m-concat along free dim:
    t_Xrep = nc.dram_tensor("Xrep", [2, 128, M * D], fp, kind="ExternalInput")
    t_Brep = nc.dram_tensor("Brep", [2, 128, M * D], fp, kind="ExternalInput")
    t_Bq = nc.dram_tensor("Bq_w", [2, M, D], fp, kind="ExternalInput")
    t_Q = nc.dram_tensor("Q_w", [2, M, 1, D], fp, kind="ExternalInput")
    t_r1 = nc.dram_tensor("r_user", [M, NLOC], i32, kind="ExternalInput")
    t_r2 = nc.dram_tensor("r_prod", [M, NLOC], i32, kind="ExternalInput")
    t_eye = nc.dram_tensor("eye128", [128, 128], fp, kind="ExternalInput")

    t_uout = nc.dram_tensor("user_out_shard", [NLOC, E], fp, kind="ExternalOutput")
    t_pout = nc.dram_tensor("product_out_shard", [NLOC, E], fp, kind="ExternalOutput")

    # softmax baseline constants (match reference semantics without max-sub)
    CB = [float((N_NODES - K) * np.exp(np.float32(-1e-9)))] + \
         [float((N_NODES - K) * np.exp(np.float32(1.0) / N_NODES))] * (M - 1)

    with tile.TileContext(nc) as tc:
        with (
            tc.tile_pool(name="wpool", bufs=1) as wp,
            tc.tile_pool(name="spool", bufs=1) as sp,
            tc.tile_pool(name="mpool", bufs=3) as mp,
            tc.tile_pool(name="aggpool", bufs=1) as agp,
            tc.tile_pool(name="psum", bufs=3, space="PSUM") as pp,
            tc.tile_pool(name="pbeta", bufs=1, space="PSUM") as pb,
            tc.tile_pool(name="dram", bufs=1, space="DRAM") as dp,
        ):
            # ---------------- persistent weights -------------------------
            eye = wp.tile([128, 128], fp, name="eye")
            nc.sync.dma_start(eye[:], t_eye[:])
            ones_r = wp.tile([1, 128], fp, name="ones_r")
            nc.vector.memset(ones_r[:], 1.0)

            Vw, Wpw, Wqw, BqT, qT = {}, {}, {}, {}, {}
            for ph in range(2):
                for m in range(M):
                    Vw[ph, m] = wp.tile([E, D], fp, name=f"V_{ph}_{m}")
                    nc.sync.dma_start(Vw[ph, m][:], t_V[ph, m])
                    Wpw[ph, m] = wp.tile([E, D], fp, name=f"Wp_{ph}_{m}")
                    nc.sync.dma_start(Wpw[ph, m][:], t_Wp[ph, m])
                    Wqw[ph, m] = wp.tile([E, D], fp, name=f"Wq_{ph}_{m}")
                    nc.sync.dma_start(Wqw[ph, m][:], t_Wq[ph, m])
                    BqT[ph, m] = wp.tile([D, 1], fp, name=f"BqT_{ph}_{m}")
                    nc.sync.dma_start(BqT[ph, m][:], t_Bq[ph, m, :, None])
                    qT[ph, m] = wp.tile([D, 1], fp, name=f"qT_{ph}_{m}")
                    nc.sync.dma_start(qT[ph, m][:], t_Q[ph, m, 0, :, None])
            x_all, b_all = {}, {}
            for ph in range(2):
                x_all[ph] = wp.tile([128, M * D], fp, name=f"xall_{ph}")
                nc.sync.dma_start(x_all[ph][:], t_Xrep[ph])
                b_all[ph] = wp.tile([128, M * D], fp, name=f"ball_{ph}")
                nc.sync.dma_start(b_all[ph][:], t_Brep[ph])

            # ---------------- internal DRAM ------------------------------
            t5e = dp.tile([N_NODES, E], fp, name="t5e")
            t5pw = [dp.tile([N_NODES, D], fp, name=f"t5pw_{m}") for m in range(M)]
            ag_in = dp.tile([NLOC, E], fp, name="ag_in")
            shared = "Shared" if CORES > 4 else "Local"
            ag_out = dp.tile([N_NODES, E], fp, name="ag_out", addr_space=shared)


            def t5_dst(t5ap, a, width):
                """AP for writing source rows [128a, 128a+128) of a table into
                its T5 layout ([r, u] -> flat row r*K + u)."""
                v = t5ap[:].rearrange("(r u) e -> r u e", u=K)
                if STRIDE >= 128:
                    r0 = (128 * a) % STRIDE
                    u0 = (128 * a) // STRIDE
                    return v[r0:r0 + 128, u0, :]
                # shrunk configs: 128 rows span several u slots
                g = 128 // STRIDE
                u0 = (128 * a) // STRIDE
                return v[:, u0:u0 + g, :].rearrange("r g e -> g r e")

            def emit_phase(ph, src_dram, other_dram, r_dram, out_drams):
                # ---- src tiles + srcT --------------------------------------
                src_sb = []
                srcT = sp.tile([128, NLOC], fp, name=f"srcT_{ph}", tag="srcT")
                for nb in range(NB):
                    st = sp.tile([128, E], fp, name=f"src_{ph}_{nb}", tag=f"src{nb}")
                    nc.sync.dma_start(st[:], src_dram[nb * 128:(nb + 1) * 128, :])
                    src_sb.append(st)
                    pt = pp.tile([128, 128], fp, name=f"pt_{ph}_{nb}", tag="pmain", space="PSUM")
                    nc.tensor.transpose(pt[:], st[:], eye[:])
                    nc.scalar.copy(srcT[:, nb * 128:(nb + 1) * 128], pt[:])

                # ---- T5 tables + PW ---------------------------------------
                for a in range(N_NODES // 128):
                    ot = mp.tile([128, E], fp, name=f"ot_{ph}_{a}", tag="ot")
                    nc.sync.dma_start(ot[:], other_dram[128 * a:128 * (a + 1), :])
                    nc.sync.dma_start(t5_dst(t5e, a, E), ot[:])
                    ptr = pp.tile([128, 128], fp, name=f"potT_{ph}_{a}", tag="pmain", space="PSUM")
                    nc.tensor.transpose(ptr[:], ot[:], eye[:])
                    otT = mp.tile([128, 128], fp, name=f"otT_{ph}_{a}", tag="otT")
                    nc.scalar.copy(otT[:], ptr[:])
                    ppw = pp.tile([128, M * D], fp, name=f"ppw_{ph}_{a}", tag="pmain", space="PSUM")
                    for m in range(M):
                        nc.tensor.matmul(ppw[:, m * D:(m + 1) * D], lhsT=otT[:],
                                         rhs=Wpw[ph, m][:], start=True, stop=True)
                    pwt = mp.tile([128, M * D], fp, name=f"pw_{ph}_{a}", tag="pwt")
                    nc.vector.tensor_copy(pwt[:], ppw[:])
                    for m in range(M):
                        nc.sync.dma_start(t5_dst(t5pw[m], a, D),
                                          pwt[:, m * D:(m + 1) * D])

                # ---- S' = src @ V + b  ([128, nb, m, D] in SBUF) -----------
                spr = sp.tile([128, NB * M * D], fp, name=f"spr_{ph}", tag="spr")
                for nb in range(NB):
                    psp = pp.tile([128, M * D], fp, name=f"psp_{ph}_{nb}", tag="pmain", space="PSUM")
                    for m in range(M):
                        nc.tensor.matmul(psp[:, m * D:(m + 1) * D],
                                         lhsT=srcT[:, nb * 128:(nb + 1) * 128],
                                         rhs=Vw[ph, m][:], start=True, stop=True)
                    nc.vector.tensor_tensor(
                        out=spr[:, nb * M * D:(nb + 1) * M * D],
                        in0=psp[:], in1=b_all[ph][:], op=ALU.add)

                # ---- main loop --------------------------------------------
                pbeta = [pb.tile([1, 128], fp, name=f"pbeta_{ph}_{m}",
                                 tag=f"pbeta{m}", space="PSUM") for m in range(M)]
                r_all = sp.tile([128, M * NB], i32, name=f"rall_{ph}", tag="rall")
                nc.sync.dma_start(
                    r_all[:],
                    r_dram[:].rearrange("m (nb p) -> p (m nb)", p=128))
                aggs = {}
                t5e_v = t5e[:].rearrange("(r u) e -> r (u e)", u=K)
                for nb in range(NB):
                    for m in range(M):
                        t5pw_v = t5pw[m][:].rearrange("(r u) d -> r (u d)", u=K)
                        rt = r_all[:, m * NB + nb:m * NB + nb + 1]
                        gpw = mp.tile([128, K * D], fp, name=f"gpw_{ph}_{nb}_{m}", tag="gpw")
                        gemb_skip = VARIANT == "no_gather"
                        if gemb_skip:
                            nc.vector.memset(gpw[:], 0.01)
                        else:
                            nc.gpsimd.indirect_dma_start(
                                out=gpw[:], out_offset=None, in_=t5pw_v,
                                in_offset=bass.IndirectOffsetOnAxis(ap=rt[:, :1], axis=0))
                        gemb = mp.tile([128, K * E], fp, name=f"ge_{ph}_{nb}_{m}", tag="gemb")
                        if gemb_skip:
                            nc.vector.memset(gemb[:], 0.01)
                        else:
                            nc.gpsimd.indirect_dma_start(
                                out=gemb[:], out_offset=None, in_=t5e_v,
                                in_offset=bass.IndirectOffsetOnAxis(ap=rt[:, :1], axis=0))
                        if VARIANT == "gather_only":
                            agg = agp.tile([128, E], fp, name=f"agg_{ph}_{nb}_{m}",
                                           tag=f"agg{nb}_{m}")
                            nc.vector.tensor_tensor(out=agg[:], in0=gemb[:, :E],
                                                    in1=gpw[:, :E], op=ALU.add)
                            aggs[nb, m] = agg
                            nc.tensor.matmul(pbeta[m][:], lhsT=qT[ph, m][:],
                                             rhs=srcT[:64, nb * 128:(nb + 1) * 128],
                                             start=(nb == 0), stop=(nb == NB - 1))
                            continue

                        spm = spr[:, (nb * M + m) * D:(nb * M + m + 1) * D]
                        h = mp.tile([128, K * D], fp, name=f"h_{ph}_{nb}_{m}", tag="h")
                        h3 = h[:].rearrange("p (k d) -> p k d", d=D)
                        nc.vector.tensor_tensor(
                            out=h3, in0=gpw[:].rearrange("p (k d) -> p k d", d=D),
                            in1=spm[:, None, :].to_broadcast([128, K, D]), op=ALU.add)
                        nc.scalar.activation(h3, h3, AF.Tanh)
                        xm = x_all[ph][:, m * D:(m + 1) * D]
                        nc.vector.tensor_tensor(
                            out=h3, in0=h3,
                            in1=xm[:, None, :].to_broadcast([128, K, D]), op=ALU.mult)
                        sc = mp.tile([128, K], fp, name=f"sc_{ph}_{nb}_{m}", tag="sc")
                        nc.vector.tensor_reduce(sc[:], h3, axis=AX.X, op=ALU.add)
                        esc = mp.tile([128, K], fp, name=f"esc_{ph}_{nb}_{m}", tag="esc")
                        den = mp.tile([128, 1], fp, name=f"den_{ph}_{nb}_{m}", tag="den")
                        nc.scalar.activation(esc[:], sc[:], AF.Exp, accum_out=den[:])
                        nc.vector.tensor_scalar_add(den[:], den[:], CB[m])
                        rin = mp.tile([128, 1], fp, name=f"rin_{ph}_{nb}_{m}", tag="rin")
                        nc.vector.reciprocal(rin[:], den[:])
                        att = mp.tile([128, K], fp, name=f"att_{ph}_{nb}_{m}", tag="att")
                        nc.vector.tensor_scalar_mul(att[:], esc[:], rin[:, :1])

                        # agg = sum_u A[:,u] * emb[:,u,:]  -> [128, E]
                        wemb = mp.tile([128, E * K], fp, name=f"we_{ph}_{nb}_{m}", tag="wemb")
                        we3 = wemb[:].rearrange("p (e k) -> p k e", k=K)
                        nc.vector.tensor_tensor(
                            out=we3, in0=gemb[:].rearrange("p (k e) -> p k e", e=E),
                            in1=att[:, :, None].to_broadcast([128, K, E]), op=ALU.mult)
                        agg = agp.tile([128, E], fp, name=f"agg_{ph}_{nb}_{m}",
                                       tag=f"agg{nb}_{m}")
                        nc.vector.tensor_reduce(
                            agg[:], wemb[:].rearrange("p (e k) -> p e k", k=K),
                            axis=AX.X, op=ALU.add)
                        aggs[nb, m] = agg

                        # sem path: semT = tanh(Wq^T @ (srcT + aggT) + BqT)
                        pat = pp.tile([128, 128], fp, name=f"pat_{ph}_{nb}_{m}", tag="pmain", space="PSUM")
                        nc.tensor.transpose(pat[:], agg[:], eye[:])
                        aggT = mp.tile([128, 128], fp, name=f"at_{ph}_{nb}_{m}", tag="aggT")
                        nc.scalar.copy(aggT[:], pat[:])
                        psem = pp.tile([D, 128], fp, name=f"ps_{ph}_{nb}_{m}", tag="pmain", space="PSUM")
                        nc.tensor.matmul(psem[:], lhsT=Wqw[ph, m][:],
                                         rhs=srcT[:, nb * 128:(nb + 1) * 128],
                                         start=True, stop=False)
                        nc.tensor.matmul(psem[:], lhsT=Wqw[ph, m][:], rhs=aggT[:],
                                         start=False, stop=True)
                        semT = mp.tile([D, 128], fp, name=f"st_{ph}_{nb}_{m}", tag="semT")
                        nc.scalar.activation(semT[:], psem[:], AF.Tanh, bias=BqT[ph, m][:, :1])
                        nc.tensor.matmul(pbeta[m][:], lhsT=qT[ph, m][:], rhs=semT[:],
                                         start=(nb == 0), stop=(nb == NB - 1))

                # ---- beta (AllReduce of partial means, then softmax) -------
                ar_in = dp.tile([1, 8], fp, name=f"ar_in_{ph}")
                ar_out = dp.tile([1, 8], fp, name=f"ar_out_{ph}", addr_space=shared)
                braw = mp.tile([1, 8], fp, name=f"braw_{ph}", tag="braw")
                nc.vector.memset(braw[:], 0.0)
                for m in range(M):
                    nc.vector.tensor_reduce(braw[:, m:m + 1], pbeta[m][:],
                                            axis=AX.X, op=ALU.add)
                nc.vector.tensor_scalar_mul(braw[:], braw[:], 1.0 / N_NODES)
                nc.gpsimd.dma_start(ar_in[:], braw[:])
                nc.gpsimd.collective_compute(
                    "AllReduce", ALU.add,
                    replica_groups=[list(range(CORES))],
                    ins=[ar_in.opt()], outs=[ar_out.opt()])
                brg = mp.tile([1, 8], fp, name=f"brg_{ph}", tag="brg")
                nc.sync.dma_start(brg[:], ar_out[:])
                eb = mp.tile([1, M], fp, name=f"eb_{ph}", tag="eb")
                ebs = mp.tile([1, 1], fp, name=f"ebs_{ph}", tag="ebs")
                nc.scalar.activation(eb[:], brg[:, :M], AF.Exp, accum_out=ebs[:])
                ebr = mp.tile([1, 1], fp, name=f"ebr_{ph}", tag="ebr")
                nc.vector.reciprocal(ebr[:], ebs[:])
                beta = mp.tile([1, M], fp, name=f"beta_{ph}", tag="beta")
                nc.vector.tensor_scalar_mul(beta[:], eb[:], ebr[:, :1])
                pbb = pp.tile([128, M], fp, name=f"pbb_{ph}", tag="pmain", space="PSUM")
                nc.tensor.matmul(pbb[:], lhsT=ones_r[:], rhs=beta[:], start=True, stop=True)
                beta_bc = mp.tile([128, M], fp, name=f"bbc_{ph}", tag="bbc")
                nc.vector.tensor_copy(beta_bc[:], pbb[:])

                # ---- out = src + sum_m beta_m * agg_m ----------------------
                for nb in range(NB):
                    out_t = mp.tile([128, E], fp, name=f"out_{ph}_{nb}", tag="outt")
                    tmp_t = mp.tile([128, E], fp, name=f"tmp_{ph}_{nb}", tag="tmpt")
                    nc.vector.tensor_scalar_mul(out_t[:], aggs[nb, 0][:], beta_bc[:, 0:1])
                    nc.vector.tensor_tensor(out=out_t[:], in0=out_t[:],
                                            in1=src_sb[nb][:], op=ALU.add)
                    for m in range(1, M):
                        nc.vector.tensor_scalar_mul(tmp_t[:], aggs[nb, m][:],
                                                    beta_bc[:, m:m + 1])
                        nc.vector.tensor_tensor(out=out_t[:], in0=out_t[:],
                                                in1=tmp_t[:], op=ALU.add)
                    for od in out_drams:
                        nc.sync.dma_start(od[nb * 128:(nb + 1) * 128, :], out_t[:])

            # ================= phase 1: users ============================
            emit_phase(0, t_user, t_prod_full, t_r1, [t_uout, ag_in])
            nc.gpsimd.collective_compute(
                "AllGather", mybir.AluOpType.bypass,
                replica_groups=[list(range(CORES))],
                ins=[ag_in.opt()], outs=[ag_out.opt()])
            # ================= phase 2: products =========================
            emit_phase(1, t_prod_shard, ag_out, t_r2, [t_pout])

    nc.compile()
    return nc


def _get_graph():
    if "nc" not in _CACHE:
        _CACHE["nc"] = _build_graph()
    return _CACHE["nc"]


# ---------------------------------------------------------------- runner
def _get_runner():
    """Build (once) a cached jitted SPMD executable for the graph.

    Mirrors concourse.bass2jax.run_bass_via_pjrt's multi-core path but keeps
    the jitted function so repeated kernel() calls don't retrace/recompile,
    and exposes device-resident timing.
    """
    if "runner" in _CACHE:
        return _CACHE["runner"]
    import sys
    if "/opt/trn_rl_repo" not in sys.path:
        sys.path.insert(0, "/opt/trn_rl_repo")
    import jax
    import numpy as _np
    from jax.experimental.shard_map import shard_map
    from jax.sharding import Mesh, PartitionSpec
    from concourse import bass2jax, mybir

    nc = _get_graph()
    bass2jax.install_neuronx_cc_hook()
    assert nc.dbg_addr is None
    pid_name = nc.partition_id_tensor.name if nc.partition_id_tensor else None

    in_names, out_names, out_avals = [], [], []
    for alloc in nc.m.functions[0].allocations:
        if not isinstance(alloc, mybir.MemoryLocationSet):
            continue
        name = alloc.memorylocations[0].name
        if alloc.kind == "ExternalInput":
            if name != pid_name:
                in_names.append(name)
        elif alloc.kind == "ExternalOutput":
            out_names.append(name)
            out_avals.append(jax.core.ShapedArray(
                tuple(alloc.tensor_shape), mybir.dt.np(alloc.dtype)))
    n_params = len(in_names)
    all_names = in_names + out_names
    if pid_name is not None:
        all_names = all_names + [pid_name]

    def _body(*args):
        operands = list(args)
        if pid_name is not None:
            operands.append(bass2jax.partition_id_tensor())
        outs = bass2jax._bass_exec_p.bind(
            *operands, out_avals=tuple(out_avals), in_names=tuple(all_names),
            out_names=tuple(out_names), lowering_input_output_aliases=(),
            sim_require_finite=True, sim_require_nnan=True, nc=nc)
        return tuple(outs)

    devices = jax.devices()[:CORES]
    mesh = Mesh(_np.asarray(devices), ("core",))
    n_outs = len(out_names)
    in_specs = (PartitionSpec("core"),) * (n_params + n_outs)
    out_specs = (PartitionSpec("core"),) * n_outs
    donate = tuple(range(n_params, n_params + n_outs))
    sharded = jax.jit(
        shard_map(_body, mesh=mesh, in_specs=in_specs, out_specs=out_specs,
                  check_rep=False),
        donate_argnums=donate, keep_unused=True)

    runner = dict(fn=sharded, in_names=in_names, out_names=out_names,
                  out_avals=out_avals, mesh=mesh)
    _CACHE["runner"] = runner
    return runner


def _run_spmd(in_maps, timeit=0):
    """Run the SPMD graph; returns (per-core results list, best_step_ns|None)."""
    import jax
    import jax.numpy as jnp
    import numpy as _np
    import time as _time
    from jax.sharding import NamedSharding, PartitionSpec

    r = _get_runner()
    fn, in_names, out_names, out_avals = \
        r["fn"], r["in_names"], r["out_names"], r["out_avals"]
    mesh = r["mesh"]

    concat_in = [_np.concatenate([_np.asarray(in_maps[c][k]) for c in range(CORES)],
                                 axis=0) for k in in_names]
    sharding = NamedSharding(mesh, PartitionSpec("core"))
    dev_in = [jax.device_put(a, sharding) for a in concat_in]

    def zeros():
        return [jax.device_put(
            _np.zeros((CORES * av.shape[0], *av.shape[1:]), av.dtype), sharding)
            for av in out_avals]

    outs = fn(*dev_in, *zeros())
    jax.block_until_ready(outs)
    best_ns = None
    if timeit:
        # Amortize the axon dispatch overhead: queue `timeit` executions
        # asynchronously, block once; subtract a single-call baseline.
        zs = [zeros() for _ in range(timeit)]
        for z in zs:
            jax.block_until_ready(z)
        t0 = _time.perf_counter()
        outs2 = fn(*dev_in, *zs[0])
        jax.block_until_ready(outs2)
        t_one = _time.perf_counter() - t0
        t0 = _time.perf_counter()
        many = [fn(*dev_in, *z) for z in zs[1:]]
        for o in many:
            jax.block_until_ready(o)
        t_many = _time.perf_counter() - t0
        per = t_many / (timeit - 1)
        best_ns = int(per * 1e9)
        print(f"[timing] single {t_one*1e3:.2f} ms, pipelined avg {per*1e3:.3f} ms")
        outs = many[-1]
    np_outs = [_np.asarray(o) for o in outs]
    results = [{name: np_outs[i].reshape(CORES, *out_avals[i].shape)[c]
                for i, name in enumerate(out_names)} for c in range(CORES)]
    return results, best_ns


def _make_in_maps(user, product, V, X, W_p, B_p, W_q, B_q, Q,
                  user_nbrs, product_nbrs):
    Xrep = np.ascontiguousarray(
        np.broadcast_to(X[:, :, 0, :][:, None, :, :], (2, 128, M, D))
        .reshape(2, 128, M * D)).astype(np.float32)
    Brep = np.ascontiguousarray(
        np.broadcast_to(B_p[:, None, :, :], (2, 128, M, D))
        .reshape(2, 128, M * D)).astype(np.float32)
    r_user = (user_nbrs[:, :, 0] % STRIDE).astype(np.int32)
    r_prod = (product_nbrs[:, :, 0] % STRIDE).astype(np.int32)
    eye = np.eye(128, dtype=np.float32)
    in_maps = []
    for c in range(CORES):
        rows = slice(c * NLOC, (c + 1) * NLOC)
        in_maps.append({
            "user_shard": user[rows],
            "product_shard": product[rows],
            "product_full": product,
            "V_w": V, "Wp_w": W_p, "Wq_w": W_q,
            "Xrep": Xrep, "Brep": Brep,
            "Bq_w": B_q, "Q_w": Q,
            "r_user": np.ascontiguousarray(r_user[:, rows]),
            "r_prod": np.ascontiguousarray(r_prod[:, rows]),
            "eye128": eye,
        })
    return in_maps


# ---------------------------------------------------------------- entry
def kernel(user, product, V, X, W_p, B_p, W_q, B_q, Q, user_nbrs, product_nbrs):
    user = np.asarray(user, np.float32)
    product = np.asarray(product, np.float32)
    V = np.asarray(V, np.float32)
    X = np.asarray(X, np.float32)
    W_p = np.asarray(W_p, np.float32)
    B_p = np.asarray(B_p, np.float32)
    W_q = np.asarray(W_q, np.float32)
    B_q = np.asarray(B_q, np.float32)
    Q = np.asarray(Q, np.float32)
    user_nbrs = np.asarray(user_nbrs)
    product_nbrs = np.asarray(product_nbrs)

    if not (_check_structured(user_nbrs) and _check_structured(product_nbrs)):
        # General-index fallback: same math on the host.
        return _reference_np(user, product, V, X, W_p, B_p, W_q, B_q, Q,
                             user_nbrs, product_nbrs)

    in_maps = _make_in_maps(user, product, V, X, W_p, B_p, W_q, B_q, Q,
                            user_nbrs, product_nbrs)
    results, _ = _run_spmd(in_maps)
    user_out = np.concatenate([results[c]["user_out_shard"]
                               for c in range(CORES)], axis=0)
    product_out = np.concatenate([results[c]["product_out_shard"]
                                  for c in range(CORES)], axis=0)
    return (user_out, product_out)


# revision 13
# speedup vs baseline: 152.2156x; 21.4260x over previous
"""Trainium2 Bass kernel for nn_AttributeEmbeddingLayer (gnn_message_passing).

Two-phase heterogeneous GNN attention layer on 8 NeuronCores:
  phase 1: user rows attend over product embeddings (user_nbrs)
  phase 2: product rows attend over the UPDATED user embeddings (product_nbrs)

Distribution: data-parallel over the node dimension (1024 rows/core), small
parameter tensors replicated, the other-type embedding table replicated
(phase 2's table is produced on-device via AllGather); the Beta reduction is
a cross-device AllReduce of 4 partial sums.

Fast path exploits the neighbor-list structure (the K=32 neighbor indices of
every node share one residue r mod (N/K), i.e. they are exactly the rows
{r + 256*u}): tables are re-laid out on device so each node's 32 neighbor
rows form ONE contiguous block, gathered with a single-index-per-partition
indirect DMA (the only gather shape TRN2 hardware supports efficiently).
The structure is verified on the host; inputs without it fall back to a
numpy implementation of the same math.
"""

import numpy as np

# ---------------------------------------------------------------- constants
N_NODES = 8192      # nodes per type (users == products == 8192)
E = 128             # embedding dim
D = 64              # attention dim
K = 32              # neighbors per (metapath, node)
M = 4               # metapaths
CORES = 8
NLOC = N_NODES // CORES          # 1024 rows per core
NB = NLOC // 128                 # 8 n-blocks of 128 rows per core
STRIDE = N_NODES // K            # 256; neighbor sets are {r + STRIDE*u}
NRES = STRIDE

FP = None  # mybir.dt.float32, set lazily
VARIANT = "full"  # "full" | "gather_only" | "no_gather"  (perf bisection)


# ---------------------------------------------------------------- host math
def _phase_np(src, other, nbrs, v, x, w, b, wq, bq, q):
    """Numpy port of the reference _phase (used as fallback / verification)."""
    m, n, k = nbrs.shape
    n_other = other.shape[0]
    out = src.copy()
    beta_raw = np.zeros(m, np.float32)
    H_all = np.empty((m, n, src.shape[1]), np.float32)
    baseline = np.where(np.arange(m) == 0, np.float32(-1e-9),
                        np.float32(1.0) / n_other).astype(np.float32)
    for mi in range(m):
        agg = np.empty((n, src.shape[1]), np.float32)
        CH = 1024
        for s in range(0, n, CH):
            sl = slice(s, s + CH)
            nbr = other[nbrs[mi, sl]]                      # [CH,K,E]
            ps = src[sl] @ v[mi]                          # [CH,D]
            pn = nbr @ w[mi]                              # [CH,K,D]
            h = np.tanh(ps[:, None, :] + pn + b[mi][None, None, :])
            sc = h @ x[mi, 0]                             # [CH,K]
            mx = np.maximum(sc.max(-1), baseline[mi])
            e = np.exp(sc - mx[:, None])
            den = e.sum(-1) + (n_other - k) * np.exp(baseline[mi] - mx)
            A = e / den[:, None]
            agg[sl] = np.einsum('nk,nke->ne', A, nbr)
        H = src + agg
        H_all[mi] = H
        sem = np.tanh(H @ wq[mi] + bq[mi][None, :])
        beta_raw[mi] = (sem @ q[mi, 0]).mean()
    eb = np.exp(beta_raw - beta_raw.max())
    beta = eb / eb.sum()
    return np.einsum('m,mne->ne', beta, H_all).astype(np.float32)


def _reference_np(user, product, V, X, W_p, B_p, W_q, B_q, Q,
                  user_nbrs, product_nbrs):
    user_out = _phase_np(user, product, user_nbrs,
                         V[0], X[0], W_p[0], B_p[0], W_q[0], B_q[0], Q[0])
    product_out = _phase_np(product, user_out, product_nbrs,
                            V[1], X[1], W_p[1], B_p[1], W_q[1], B_q[1], Q[1])
    return (user_out, product_out)


def _check_structured(nbrs):
    """True iff every (m, n) neighbor set is exactly {r + STRIDE*u, u=0..K-1}."""
    if nbrs.shape != (M, N_NODES, K):
        return False
    r = nbrs[:, :, 0] % STRIDE
    want = r[:, :, None] + STRIDE * np.arange(K, dtype=nbrs.dtype)[None, None, :]
    return bool(np.array_equal(np.sort(nbrs, axis=-1), np.sort(want, axis=-1)))


# ---------------------------------------------------------------- device IR
_CACHE = {}


def _build_graph():
    import sys
    if "/opt/trn_rl_repo" not in sys.path:
        sys.path.insert(0, "/opt/trn_rl_repo")
    import concourse.bass as bass
    import concourse.bacc as bacc
    import concourse.mybir as mybir
    import concourse.tile as tile

    fp = mybir.dt.float32
    bf = mybir.dt.bfloat16
    i32 = mybir.dt.int32
    AF = mybir.ActivationFunctionType
    ALU = mybir.AluOpType
    AX = mybir.AxisListType

    nc = bacc.Bacc("TRN2", target_bir_lowering=False, num_devices=CORES)

    # ---------------- I/O -------------------------------------------------
    t_user = nc.dram_tensor("user_shard", [NLOC, E], fp, kind="ExternalInput")
    t_prod_shard = nc.dram_tensor("product_shard", [NLOC, E], fp, kind="ExternalInput")
    t_prod_full = nc.dram_tensor("product_full", [N_NODES, E], fp, kind="ExternalInput")
    t_V = nc.dram_tensor("V_w", [2, M, E, D], fp, kind="ExternalInput")
    t_Wp = nc.dram_tensor("Wp_w", [2, M, E, D], fp, kind="ExternalInput")
    t_Wq = nc.dram_tensor("Wq_w", [2, M, E, D], fp, kind="ExternalInput")
    # host-replicated across 128 partitions, m-concat along free dim:
    t_Xrep = nc.dram_tensor("Xrep", [2, 128, M * D], fp, kind="ExternalInput")
    t_Brep = nc.dram_tensor("Brep", [2, 128, M * D], fp, kind="ExternalInput")
    t_Bq = nc.dram_tensor("Bq_w", [2, M, D], fp, kind="ExternalInput")
    t_Q = nc.dram_tensor("Q_w", [2, M, 1, D], fp, kind="ExternalInput")
    t_r1 = nc.dram_tensor("r_user", [M, NLOC], i32, kind="ExternalInput")
    t_r2 = nc.dram_tensor("r_prod", [M, NLOC], i32, kind="ExternalInput")
    t_eye = nc.dram_tensor("eye128", [128, 128], fp, kind="ExternalInput")

    t_uout = nc.dram_tensor("user_out_shard", [NLOC, E], fp, kind="ExternalOutput")
    t_pout = nc.dram_tensor("product_out_shard", [NLOC, E], fp, kind="ExternalOutput")

    # softmax baseline constants (match reference semantics without max-sub)
    CB = [float((N_NODES - K) * np.exp(np.float32(-1e-9)))] + \
         [float((N_NODES - K) * np.exp(np.float32(1.0) / N_NODES))] * (M - 1)

    with tile.TileContext(nc) as tc:
        with (
            tc.tile_pool(name="wpool", bufs=1) as wp,
            tc.tile_pool(name="spool", bufs=1) as sp,
            tc.tile_pool(name="mpool", bufs=3) as mp,
            tc.tile_pool(name="aggpool", bufs=1) as agp,
            tc.tile_pool(name="psum", bufs=3, space="PSUM") as pp,
            tc.tile_pool(name="pbeta", bufs=1, space="PSUM") as pb,
            tc.tile_pool(name="dram", bufs=1, space="DRAM") as dp,
        ):
            # ---------------- persistent weights -------------------------
            eye = wp.tile([128, 128], fp, name="eye")
            nc.sync.dma_start(eye[:], t_eye[:])
            ones_r = wp.tile([1, 128], fp, name="ones_r")
            nc.vector.memset(ones_r[:], 1.0)

            Vw, Wpw, Wqw, BqT, qT = {}, {}, {}, {}, {}
            for ph in range(2):
                for m in range(M):
                    Vw[ph, m] = wp.tile([E, D], fp, name=f"V_{ph}_{m}")
                    nc.sync.dma_start(Vw[ph, m][:], t_V[ph, m])
                    Wpw[ph, m] = wp.tile([E, D], fp, name=f"Wp_{ph}_{m}")
                    nc.sync.dma_start(Wpw[ph, m][:], t_Wp[ph, m])
                    Wqw[ph, m] = wp.tile([E, D], fp, name=f"Wq_{ph}_{m}")
                    nc.sync.dma_start(Wqw[ph, m][:], t_Wq[ph, m])
                    BqT[ph, m] = wp.tile([D, 1], fp, name=f"BqT_{ph}_{m}")
                    nc.sync.dma_start(BqT[ph, m][:], t_Bq[ph, m, :, None])
                    qT[ph, m] = wp.tile([D, 1], fp, name=f"qT_{ph}_{m}")
                    nc.sync.dma_start(qT[ph, m][:], t_Q[ph, m, 0, :, None])
            x_all, b_all = {}, {}
            for ph in range(2):
                x_all[ph] = wp.tile([128, M * D], bf, name=f"xall_{ph}")
                nc.gpsimd.dma_start(x_all[ph][:], t_Xrep[ph])
                b_all[ph] = wp.tile([128, M * D], fp, name=f"ball_{ph}")
                nc.sync.dma_start(b_all[ph][:], t_Brep[ph])

            # ---------------- internal DRAM ------------------------------
            t5e = dp.tile([N_NODES, E], bf, name="t5e")
            t5pw = [dp.tile([N_NODES, D], bf, name=f"t5pw_{m}") for m in range(M)]
            ag_in = dp.tile([NLOC, E], fp, name="ag_in")
            shared = "Shared" if CORES > 4 else "Local"
            ag_out = dp.tile([N_NODES, E], fp, name="ag_out", addr_space=shared)


            def t5_dst(t5ap, a, width):
                """AP for writing source rows [128a, 128a+128) of a table into
                its T5 layout ([r, u] -> flat row r*K + u)."""
                v = t5ap[:].rearrange("(r u) e -> r u e", u=K)
                if STRIDE >= 128:
                    r0 = (128 * a) % STRIDE
                    u0 = (128 * a) // STRIDE
                    return v[r0:r0 + 128, u0, :]
                # shrunk configs: 128 rows span several u slots
                g = 128 // STRIDE
                u0 = (128 * a) // STRIDE
                return v[:, u0:u0 + g, :].rearrange("r g e -> g r e")

            def emit_phase(ph, src_dram, other_dram, r_dram, out_drams):
                # ---- src tiles + srcT --------------------------------------
                src_sb = []
                srcT = sp.tile([128, NLOC], fp, name=f"srcT_{ph}", tag="srcT")
                for nb in range(NB):
                    st = sp.tile([128, E], fp, name=f"src_{ph}_{nb}", tag=f"src{nb}")
                    nc.sync.dma_start(st[:], src_dram[nb * 128:(nb + 1) * 128, :])
                    src_sb.append(st)
                    pt = pp.tile([128, 128], fp, name=f"pt_{ph}_{nb}", tag="pmain", space="PSUM")
                    nc.tensor.transpose(pt[:], st[:], eye[:])
                    nc.scalar.copy(srcT[:, nb * 128:(nb + 1) * 128], pt[:])

                # ---- T5 tables + PW ---------------------------------------
                for a in range(N_NODES // 128):
                    ot = mp.tile([128, E], fp, name=f"ot_{ph}_{a}", tag="ot")
                    nc.sync.dma_start(ot[:], other_dram[128 * a:128 * (a + 1), :])
                    nc.gpsimd.dma_start(t5_dst(t5e, a, E), ot[:])
                    ptr = pp.tile([128, 128], fp, name=f"potT_{ph}_{a}", tag="pmain", space="PSUM")
                    nc.tensor.transpose(ptr[:], ot[:], eye[:])
                    otT = mp.tile([128, 128], fp, name=f"otT_{ph}_{a}", tag="otT")
                    nc.scalar.copy(otT[:], ptr[:])
                    ppw = pp.tile([128, M * D], fp, name=f"ppw_{ph}_{a}", tag="pmain", space="PSUM")
                    for m in range(M):
                        nc.tensor.matmul(ppw[:, m * D:(m + 1) * D], lhsT=otT[:],
                                         rhs=Wpw[ph, m][:], start=True, stop=True)
                    pwt = mp.tile([128, M * D], fp, name=f"pw_{ph}_{a}", tag="pwt")
                    nc.vector.tensor_copy(pwt[:], ppw[:])
                    for m in range(M):
                        nc.gpsimd.dma_start(t5_dst(t5pw[m], a, D),
                                            pwt[:, m * D:(m + 1) * D])

                # ---- S' = src @ V + b  ([128, nb, m, D] in SBUF) -----------
                spr = sp.tile([128, NB * M * D], bf, name=f"spr_{ph}", tag="spr")
                for nb in range(NB):
                    psp = pp.tile([128, M * D], fp, name=f"psp_{ph}_{nb}", tag="pmain", space="PSUM")
                    for m in range(M):
                        nc.tensor.matmul(psp[:, m * D:(m + 1) * D],
                                         lhsT=srcT[:, nb * 128:(nb + 1) * 128],
                                         rhs=Vw[ph, m][:], start=True, stop=True)
                    nc.vector.tensor_tensor(
                        out=spr[:, nb * M * D:(nb + 1) * M * D],
                        in0=psp[:], in1=b_all[ph][:], op=ALU.add)

                # ---- main loop --------------------------------------------
                pbeta = [pb.tile([1, 128], fp, name=f"pbeta_{ph}_{m}",
                                 tag=f"pbeta{m}", space="PSUM") for m in range(M)]
                r_all = sp.tile([128, M * NB], i32, name=f"rall_{ph}", tag="rall")
                nc.sync.dma_start(
                    r_all[:],
                    r_dram[:].rearrange("m (nb p) -> p (m nb)", p=128))
                aggs = {}
                t5e_v = t5e[:].rearrange("(r u) e -> r (u e)", u=K)
                for nb in range(NB):
                    for m in range(M):
                        t5pw_v = t5pw[m][:].rearrange("(r u) d -> r (u d)", u=K)
                        rt = r_all[:, m * NB + nb:m * NB + nb + 1]
                        gpw = mp.tile([128, K * D], bf, name=f"gpw_{ph}_{nb}_{m}", tag="gpw")
                        gemb_skip = VARIANT == "no_gather"
                        if gemb_skip:
                            nc.vector.memset(gpw[:], 0.01)
                        else:
                            nc.gpsimd.indirect_dma_start(
                                out=gpw[:], out_offset=None, in_=t5pw_v,
                                in_offset=bass.IndirectOffsetOnAxis(ap=rt[:, :1], axis=0))
                        gemb = mp.tile([128, K * E], bf, name=f"ge_{ph}_{nb}_{m}", tag="gemb")
                        if gemb_skip:
                            nc.vector.memset(gemb[:], 0.01)
                        else:
                            nc.gpsimd.indirect_dma_start(
                                out=gemb[:], out_offset=None, in_=t5e_v,
                                in_offset=bass.IndirectOffsetOnAxis(ap=rt[:, :1], axis=0))
                        if VARIANT == "gather_only":
                            agg = agp.tile([128, E], fp, name=f"agg_{ph}_{nb}_{m}",
                                           tag=f"agg{nb}_{m}")
                            nc.vector.tensor_tensor(out=agg[:], in0=gemb[:, :E],
                                                    in1=gpw[:, :E], op=ALU.add)
                            aggs[nb, m] = agg
                            nc.tensor.matmul(pbeta[m][:], lhsT=qT[ph, m][:],
                                             rhs=srcT[:64, nb * 128:(nb + 1) * 128],
                                             start=(nb == 0), stop=(nb == NB - 1))
                            continue

                        spm = spr[:, (nb * M + m) * D:(nb * M + m + 1) * D]
                        h = mp.tile([128, K * D], bf, name=f"h_{ph}_{nb}_{m}", tag="h")
                        h3 = h[:].rearrange("p (k d) -> p k d", d=D)
                        nc.vector.tensor_tensor(
                            out=h3, in0=gpw[:].rearrange("p (k d) -> p k d", d=D),
                            in1=spm[:, None, :].to_broadcast([128, K, D]), op=ALU.add)
                        nc.scalar.activation(h3, h3, AF.Tanh)
                        xm = x_all[ph][:, m * D:(m + 1) * D]
                        nc.vector.tensor_tensor(
                            out=h3, in0=h3,
                            in1=xm[:, None, :].to_broadcast([128, K, D]), op=ALU.mult)
                        sc = mp.tile([128, K], fp, name=f"sc_{ph}_{nb}_{m}", tag="sc")
                        nc.vector.tensor_reduce(sc[:], h3, axis=AX.X, op=ALU.add)
                        esc = mp.tile([128, K], fp, name=f"esc_{ph}_{nb}_{m}", tag="esc")
                        den = mp.tile([128, 1], fp, name=f"den_{ph}_{nb}_{m}", tag="den")
                        nc.scalar.activation(esc[:], sc[:], AF.Exp, accum_out=den[:])
                        nc.vector.tensor_scalar_add(den[:], den[:], CB[m])
                        rin = mp.tile([128, 1], fp, name=f"rin_{ph}_{nb}_{m}", tag="rin")
                        nc.vector.reciprocal(rin[:], den[:])
                        att = mp.tile([128, K], bf, name=f"att_{ph}_{nb}_{m}", tag="att")
                        nc.vector.tensor_scalar_mul(att[:], esc[:], rin[:, :1])

                        # agg = sum_u A[:,u] * emb[:,u,:]  -> [128, E]
                        wemb = mp.tile([128, E * K], bf, name=f"we_{ph}_{nb}_{m}", tag="wemb")
                        we3 = wemb[:].rearrange("p (e k) -> p k e", k=K)
                        nc.vector.tensor_tensor(
                            out=we3, in0=gemb[:].rearrange("p (k e) -> p k e", e=E),
                            in1=att[:, :, None].to_broadcast([128, K, E]), op=ALU.mult)
                        agg = agp.tile([128, E], fp, name=f"agg_{ph}_{nb}_{m}",
                                       tag=f"agg{nb}_{m}")
                        nc.vector.tensor_reduce(
                            agg[:], wemb[:].rearrange("p (e k) -> p e k", k=K),
                            axis=AX.X, op=ALU.add)
                        aggs[nb, m] = agg

                        # sem path: semT = tanh(Wq^T @ (srcT + aggT) + BqT)
                        pat = pp.tile([128, 128], fp, name=f"pat_{ph}_{nb}_{m}", tag="pmain", space="PSUM")
                        nc.tensor.transpose(pat[:], agg[:], eye[:])
                        aggT = mp.tile([128, 128], fp, name=f"at_{ph}_{nb}_{m}", tag="aggT")
                        nc.scalar.copy(aggT[:], pat[:])
                        psem = pp.tile([D, 128], fp, name=f"ps_{ph}_{nb}_{m}", tag="pmain", space="PSUM")
                        nc.tensor.matmul(psem[:], lhsT=Wqw[ph, m][:],
                                         rhs=srcT[:, nb * 128:(nb + 1) * 128],
                                         start=True, stop=False)
                        nc.tensor.matmul(psem[:], lhsT=Wqw[ph, m][:], rhs=aggT[:],
                                         start=False, stop=True)
                        semT = mp.tile([D, 128], fp, name=f"st_{ph}_{nb}_{m}", tag="semT")
                        nc.scalar.activation(semT[:], psem[:], AF.Tanh, bias=BqT[ph, m][:, :1])
                        nc.tensor.matmul(pbeta[m][:], lhsT=qT[ph, m][:], rhs=semT[:],
                                         start=(nb == 0), stop=(nb == NB - 1))

                # ---- beta (AllReduce of partial means, then softmax) -------
                ar_in = dp.tile([1, 8], fp, name=f"ar_in_{ph}")
                ar_out = dp.tile([1, 8], fp, name=f"ar_out_{ph}", addr_space=shared)
                braw = mp.tile([1, 8], fp, name=f"braw_{ph}", tag="braw")
                nc.vector.memset(braw[:], 0.0)
                for m in range(M):
                    nc.vector.tensor_reduce(braw[:, m:m + 1], pbeta[m][:],
                                            axis=AX.X, op=ALU.add)
                nc.vector.tensor_scalar_mul(braw[:], braw[:], 1.0 / N_NODES)
                nc.gpsimd.dma_start(ar_in[:], braw[:])
                nc.gpsimd.collective_compute(
                    "AllReduce", ALU.add,
                    replica_groups=[list(range(CORES))],
                    ins=[ar_in.opt()], outs=[ar_out.opt()])
                brg = mp.tile([1, 8], fp, name=f"brg_{ph}", tag="brg")
                nc.sync.dma_start(brg[:], ar_out[:])
                eb = mp.tile([1, M], fp, name=f"eb_{ph}", tag="eb")
                ebs = mp.tile([1, 1], fp, name=f"ebs_{ph}", tag="ebs")
                nc.scalar.activation(eb[:], brg[:, :M], AF.Exp, accum_out=ebs[:])
                ebr = mp.tile([1, 1], fp, name=f"ebr_{ph}", tag="ebr")
                nc.vector.reciprocal(ebr[:], ebs[:])
                beta = mp.tile([1, M], fp, name=f"beta_{ph}", tag="beta")
                nc.vector.tensor_scalar_mul(beta[:], eb[:], ebr[:, :1])
                pbb = pp.tile([128, M], fp, name=f"pbb_{ph}", tag="pmain", space="PSUM")
                nc.tensor.matmul(pbb[:], lhsT=ones_r[:], rhs=beta[:], start=True, stop=True)
                beta_bc = mp.tile([128, M], fp, name=f"bbc_{ph}", tag="bbc")
                nc.vector.tensor_copy(beta_bc[:], pbb[:])

                # ---- out = src + sum_m beta_m * agg_m ----------------------
                for nb in range(NB):
                    out_t = mp.tile([128, E], fp, name=f"out_{ph}_{nb}", tag="outt")
                    tmp_t = mp.tile([128, E], fp, name=f"tmp_{ph}_{nb}", tag="tmpt")
                    nc.vector.tensor_scalar_mul(out_t[:], aggs[nb, 0][:], beta_bc[:, 0:1])
                    nc.vector.tensor_tensor(out=out_t[:], in0=out_t[:],
                                            in1=src_sb[nb][:], op=ALU.add)
                    for m in range(1, M):
                        nc.vector.tensor_scalar_mul(tmp_t[:], aggs[nb, m][:],
                                                    beta_bc[:, m:m + 1])
                        nc.vector.tensor_tensor(out=out_t[:], in0=out_t[:],
                                                in1=tmp_t[:], op=ALU.add)
                    for od in out_drams:
                        nc.sync.dma_start(od[nb * 128:(nb + 1) * 128, :], out_t[:])

            # ================= phase 1: users ============================
            emit_phase(0, t_user, t_prod_full, t_r1, [t_uout, ag_in])
            nc.gpsimd.collective_compute(
                "AllGather", mybir.AluOpType.bypass,
                replica_groups=[list(range(CORES))],
                ins=[ag_in.opt()], outs=[ag_out.opt()])
            # ================= phase 2: products =========================
            emit_phase(1, t_prod_shard, ag_out, t_r2, [t_pout])

    nc.compile()
    return nc


def _get_graph():
    if "nc" not in _CACHE:
        _CACHE["nc"] = _build_graph()
    return _CACHE["nc"]


# ---------------------------------------------------------------- runner
def _get_runner():
    """Build (once) a cached jitted SPMD executable for the graph.

    Mirrors concourse.bass2jax.run_bass_via_pjrt's multi-core path but keeps
    the jitted function so repeated kernel() calls don't retrace/recompile,
    and exposes device-resident timing.
    """
    if "runner" in _CACHE:
        return _CACHE["runner"]
    import sys
    if "/opt/trn_rl_repo" not in sys.path:
        sys.path.insert(0, "/opt/trn_rl_repo")
    import jax
    import numpy as _np
    from jax.experimental.shard_map import shard_map
    from jax.sharding import Mesh, PartitionSpec
    from concourse import bass2jax, mybir

    nc = _get_graph()
    bass2jax.install_neuronx_cc_hook()
    assert nc.dbg_addr is None
    pid_name = nc.partition_id_tensor.name if nc.partition_id_tensor else None

    in_names, out_names, out_avals = [], [], []
    for alloc in nc.m.functions[0].allocations:
        if not isinstance(alloc, mybir.MemoryLocationSet):
            continue
        name = alloc.memorylocations[0].name
        if alloc.kind == "ExternalInput":
            if name != pid_name:
                in_names.append(name)
        elif alloc.kind == "ExternalOutput":
            out_names.append(name)
            out_avals.append(jax.core.ShapedArray(
                tuple(alloc.tensor_shape), mybir.dt.np(alloc.dtype)))
    n_params = len(in_names)
    all_names = in_names + out_names
    if pid_name is not None:
        all_names = all_names + [pid_name]

    def _body(*args):
        operands = list(args)
        if pid_name is not None:
            operands.append(bass2jax.partition_id_tensor())
        outs = bass2jax._bass_exec_p.bind(
            *operands, out_avals=tuple(out_avals), in_names=tuple(all_names),
            out_names=tuple(out_names), lowering_input_output_aliases=(),
            sim_require_finite=True, sim_require_nnan=True, nc=nc)
        return tuple(outs)

    devices = jax.devices()[:CORES]
    mesh = Mesh(_np.asarray(devices), ("core",))
    n_outs = len(out_names)
    in_specs = (PartitionSpec("core"),) * (n_params + n_outs)
    out_specs = (PartitionSpec("core"),) * n_outs
    donate = tuple(range(n_params, n_params + n_outs))
    sharded = jax.jit(
        shard_map(_body, mesh=mesh, in_specs=in_specs, out_specs=out_specs,
                  check_rep=False),
        donate_argnums=donate, keep_unused=True)

    runner = dict(fn=sharded, in_names=in_names, out_names=out_names,
                  out_avals=out_avals, mesh=mesh)
    _CACHE["runner"] = runner
    return runner


def _run_spmd(in_maps, timeit=0):
    """Run the SPMD graph; returns (per-core results list, best_step_ns|None)."""
    import jax
    import jax.numpy as jnp
    import numpy as _np
    import time as _time
    from jax.sharding import NamedSharding, PartitionSpec

    r = _get_runner()
    fn, in_names, out_names, out_avals = \
        r["fn"], r["in_names"], r["out_names"], r["out_avals"]
    mesh = r["mesh"]

    concat_in = [_np.concatenate([_np.asarray(in_maps[c][k]) for c in range(CORES)],
                                 axis=0) for k in in_names]
    sharding = NamedSharding(mesh, PartitionSpec("core"))
    dev_in = [jax.device_put(a, sharding) for a in concat_in]

    def zeros():
        return [jax.device_put(
            _np.zeros((CORES * av.shape[0], *av.shape[1:]), av.dtype), sharding)
            for av in out_avals]

    outs = fn(*dev_in, *zeros())
    jax.block_until_ready(outs)
    best_ns = None
    if timeit:
        # Amortize the axon dispatch overhead: queue `timeit` executions
        # asynchronously, block once; subtract a single-call baseline.
        zs = [zeros() for _ in range(timeit)]
        for z in zs:
            jax.block_until_ready(z)
        t0 = _time.perf_counter()
        outs2 = fn(*dev_in, *zs[0])
        jax.block_until_ready(outs2)
        t_one = _time.perf_counter() - t0
        t0 = _time.perf_counter()
        many = [fn(*dev_in, *z) for z in zs[1:]]
        for o in many:
            jax.block_until_ready(o)
        t_many = _time.perf_counter() - t0
        per = t_many / (timeit - 1)
        best_ns = int(per * 1e9)
        print(f"[timing] single {t_one*1e3:.2f} ms, pipelined avg {per*1e3:.3f} ms")
        outs = many[-1]
    np_outs = [_np.asarray(o) for o in outs]
    results = [{name: np_outs[i].reshape(CORES, *out_avals[i].shape)[c]
                for i, name in enumerate(out_names)} for c in range(CORES)]
    return results, best_ns


def _make_in_maps(user, product, V, X, W_p, B_p, W_q, B_q, Q,
                  user_nbrs, product_nbrs):
    Xrep = np.ascontiguousarray(
        np.broadcast_to(X[:, :, 0, :][:, None, :, :], (2, 128, M, D))
        .reshape(2, 128, M * D)).astype(np.float32)
    Brep = np.ascontiguousarray(
        np.broadcast_to(B_p[:, None, :, :], (2, 128, M, D))
        .reshape(2, 128, M * D)).astype(np.float32)
    r_user = (user_nbrs[:, :, 0] % STRIDE).astype(np.int32)
    r_prod = (product_nbrs[:, :, 0] % STRIDE).astype(np.int32)
    eye = np.eye(128, dtype=np.float32)
    in_maps = []
    for c in range(CORES):
        rows = slice(c * NLOC, (c + 1) * NLOC)
        in_maps.append({
            "user_shard": user[rows],
            "product_shard": product[rows],
            "product_full": product,
            "V_w": V, "Wp_w": W_p, "Wq_w": W_q,
            "Xrep": Xrep, "Brep": Brep,
            "Bq_w": B_q, "Q_w": Q,
            "r_user": np.ascontiguousarray(r_user[:, rows]),
            "r_prod": np.ascontiguousarray(r_prod[:, rows]),
            "eye128": eye,
        })
    return in_maps


# ---------------------------------------------------------------- entry
def kernel(user, product, V, X, W_p, B_p, W_q, B_q, Q, user_nbrs, product_nbrs):
    user = np.asarray(user, np.float32)
    product = np.asarray(product, np.float32)
    V = np.asarray(V, np.float32)
    X = np.asarray(X, np.float32)
    W_p = np.asarray(W_p, np.float32)
    B_p = np.asarray(B_p, np.float32)
    W_q = np.asarray(W_q, np.float32)
    B_q = np.asarray(B_q, np.float32)
    Q = np.asarray(Q, np.float32)
    user_nbrs = np.asarray(user_nbrs)
    product_nbrs = np.asarray(product_nbrs)

    if not (_check_structured(user_nbrs) and _check_structured(product_nbrs)):
        # General-index fallback: same math on the host.
        return _reference_np(user, product, V, X, W_p, B_p, W_q, B_q, Q,
                             user_nbrs, product_nbrs)

    in_maps = _make_in_maps(user, product, V, X, W_p, B_p, W_q, B_q, Q,
                            user_nbrs, product_nbrs)
    results, _ = _run_spmd(in_maps)
    user_out = np.concatenate([results[c]["user_out_shard"]
                               for c in range(CORES)], axis=0)
    product_out = np.concatenate([results[c]["product_out_shard"]
                                  for c in range(CORES)], axis=0)
    return (user_out, product_out)
